# revision 31
# baseline (speedup 1.0000x reference)
"""Trainium2 Bass kernel for GCNN operator:
    h   = einsum('bnf,nfg->bng', x, kernel)   # per-node feature transform
    out = einsum('nm,bmg->bng', A, h) + bias  # dense adjacency aggregation

Sharding: node dim N row-sharded across 8 cores for the A@h matmul only.
Every core redundantly computes the FULL h on its DVE (x and kernel are
small), so there is NO collective at all — no cross-core barrier, no skew
sensitivity. The A-shard (pre-transposed, pre-tiled and cast to fp16 on
host so every DMA descriptor is one 16KB contiguous run) streams from HBM
while the TensorEngine accumulates out^T = sum_m H_m^T @ A^T_m. Bias is
added during the PSUM->SBUF drain; the host undoes the out^T layout.

Self-contained: hardcodes shapes; only imports concourse + numpy.
"""

import numpy as np

B, N, F, G = 2, 16384, 16, 16
NCORES = 8
P = 128                    # SBUF partitions
C = B * G                  # 32 fused (batch, out-feature) columns
NT = 512                   # matmul moving-operand free-dim per instruction
KM = 8                     # contraction j-blocks per A-stream DMA
JJ = 8                     # j-blocks per H compute chunk
AT_BUFS = 3                # A-stream double buffering depth


def build_nc(n=N, ncores=NCORES, at_bufs=AT_BUFS):
    """Build the per-core Bass program (SPMD: same program on all cores)."""
    import concourse.bass as bass
    import concourse.mybir as mybir
    import concourse.tile as tile
    from concourse import bacc

    f32 = mybir.dt.float32
    f16 = mybir.dt.float16

    nl = n // ncores           # local output rows per core
    jn = n // P                # contraction j-blocks over FULL n
    km = min(KM, jn)           # j-blocks per A DMA
    mb_n = jn // km            # A-stream DMA count
    ntc = min(NT, nl)          # matmul moving free-dim
    nt_n = max(nl // ntc, 1)   # acc tiles

    # A-stream tile plan: full km-sized tiles, with the final tile split
    # into two halves so the PE's cold trail after the last byte lands is
    # halved. (o, sz, ring): ring-balanced at exactly jn/2 j-blocks each —
    # full tiles alternate rings, both halves go to ring 1.
    tile_plan = []
    for i in range(mb_n - 1):
        tile_plan.append((i * km, km, i % 2))
    o_last = (mb_n - 1) * km
    if mb_n >= 2 and km >= 2:
        tile_plan.append((o_last, km // 2, 1))
        tile_plan.append((o_last + km // 2, km - km // 2, 1))
    else:
        tile_plan.append((o_last, km, (mb_n - 1) % 2))

    # Uniform H chunks. (Small leading chunks + PE warm-up priming were
    # tried to start the PE at ~16us instead of ~40us — both regressed:
    # the end time is pinned by the saturated A-stream, and an early PE
    # start couples the at-slot recycling to the DVE pace, starving the
    # DMA rings. The late dense start keeps the rings 96% busy.)
    jj = min(JJ, jn)
    ch_sizes = [jj] * (jn // jj)
    nch = len(ch_sizes)
    ch_off = [0]
    for sz in ch_sizes:
        ch_off.append(ch_off[-1] + sz)
    # m (j-block index) -> (chunk index, local offset)
    m2ch = []
    for i, sz in enumerate(ch_sizes):
        for jl in range(sz):
            m2ch.append((i, jl))

    nc = bacc.Bacc("TRN2", target_bir_lowering=False, debug=False, num_devices=1)

    at = nc.dram_tensor("at", [P, jn, nl], f16, kind="ExternalInput")
    xq = nc.dram_tensor("xq", [P, jn * B * F], f16, kind="ExternalInput")
    kq = nc.dram_tensor("kq", [P, jn * G * F], f16, kind="ExternalInput")
    bsT = nc.dram_tensor("bsT", [C, nl], f16, kind="ExternalInput")
    outs = nc.dram_tensor("outs", [C, nl], f32, kind="ExternalOutput")

    with tile.TileContext(nc) as tc:
        with (
            tc.tile_pool(name="const", bufs=1) as const,
            tc.tile_pool(name="work", bufs=2) as work,
            tc.tile_pool(name="atp", bufs=at_bufs) as atp,
            tc.tile_pool(name="pacc", bufs=1, space="PSUM") as pacc,
        ):
            # ---- prologue loads on the two HWDGE rings, ahead of the A
            # stream in each ring's FIFO: the ~9.4MB of ks/xs/bias uses the
            # early-window bandwidth while the PE ramps, and the at tiles
            # behind them never stall the rings on slot WAR. ----
            ksq = []
            for q in range(nch):
                kt = const.tile(
                    [P, ch_sizes[q], G, F], f16, tag=f"ks{q}", name=f"ks{q}"
                )
                ksq.append(kt)
            xs = const.tile([P, jn, B, F], f16)
            biasT = const.tile([C, nl], f16)

            kq_r = kq.ap().rearrange("p (j g f) -> p j g f", g=G, f=F)
            nc.scalar.dma_start(
                out=ksq[0][:, :, :, :], in_=kq_r[:, 0 : ch_off[1]]
            )
            nc.sync.dma_start(
                out=xs[:, :, :, :],
                in_=xq.ap().rearrange("p (j b f) -> p j b f", b=B, f=F),
            )
            for q in range(1, nch):
                eng = nc.scalar if q % 2 == 0 else nc.sync
                eng.dma_start(
                    out=ksq[q][:, :, :, :], in_=kq_r[:, ch_off[q] : ch_off[q + 1]]
                )
            nc.sync.dma_start(out=biasT[:, :], in_=bsT.ap())

            # ---- full H on DVE, chunked: hq_q[p, j, (b g)] = sum_f x*k ----
            hqs = []
            with nc.allow_low_precision(reason="h accum over F=16 in fp16"):
                for q in range(nch):
                    sz = ch_sizes[q]
                    hq = const.tile([P, sz, C], f16, tag=f"hq{q}", name=f"hq{q}")
                    for b in range(B):
                        prod = work.tile([P, JJ, G, F], f16, tag="prod")
                        nc.vector.tensor_tensor(
                            prod[:, :sz, :, :],
                            xs[
                                :, ch_off[q] : ch_off[q + 1], b, None, :
                            ].to_broadcast([P, sz, G, F]),
                            ksq[q][:, :, :, :],
                            mybir.AluOpType.mult,
                        )
                        nc.vector.tensor_reduce(
                            hq[:, :, b * G : (b + 1) * G],
                            prod[:, :sz, :, :],
                            axis=mybir.AxisListType.X,
                            op=mybir.AluOpType.add,
                        )
                    hqs.append(hq)

            # ---- main loop: out^T[c, nl] += H_m^T-block @ A^T tile ----
            acc = [
                pacc.tile([C, ntc], f32, tag=f"acc{t}", name=f"acc{t}")
                for t in range(nt_n)
            ]
            outT = work.tile([C, nl], f32, tag="outT")
            rings = [nc.sync, nc.scalar]
            for ti, (o, sz, ring) in enumerate(tile_plan):
                at_t = atp.tile([P, sz, nl], f16, tag="at_t", name="at_t")
                rings[ring].dma_start(
                    out=at_t[:, :, :], in_=at.ap()[:, o : o + sz, :]
                )
                if ti < len(tile_plan) - 1:
                    for kk in range(sz):
                        m = o + kk
                        q, jl = m2ch[m]
                        for t in range(nt_n):
                            nc.tensor.matmul(
                                acc[t][:, :],
                                hqs[q][:, jl, :],
                                at_t[:, kk, t * ntc : (t + 1) * ntc],
                                start=(m == 0),
                                stop=False,
                            )
                else:
                    # Final tile: t-outer so each acc closes in turn; drain
                    # each to SBUF with a fused bias add and store it while
                    # the PE still works on the later t slices.
                    for t in range(nt_n):
                        for kk in range(sz):
                            m = o + kk
                            q, jl = m2ch[m]
                            nc.tensor.matmul(
                                acc[t][:, :],
                                hqs[q][:, jl, :],
                                at_t[:, kk, t * ntc : (t + 1) * ntc],
                                start=(m == 0),
                                stop=(kk == sz - 1),
                            )
                        nc.vector.tensor_add(
                            outT[:, t * ntc : (t + 1) * ntc],
                            acc[t][:, :],
                            biasT[:, t * ntc : (t + 1) * ntc],
                        )
                        eng2 = nc.scalar if t % 2 else nc.sync
                        eng2.dma_start(
                            out=outs.ap()[:, t * ntc : (t + 1) * ntc],
                            in_=outT[:, t * ntc : (t + 1) * ntc],
                        )

    nc.compile()
    return nc


_NC_CACHE = {}


def _get_nc(n=N, ncores=NCORES):
    key = (n, ncores)
    if key not in _NC_CACHE:
        _NC_CACHE[key] = build_nc(n, ncores)
    return _NC_CACHE[key]


def make_in_maps(x, A, kern, bias, n=N, ncores=NCORES):
    nl = n // ncores
    jn = n // P
    km = min(KM, jn)
    mb_n = jn // km

    # Shared across cores: x and kernel in [p-major] DVE-friendly layouts.
    # xq[p, j, b, f] = x[b, j*P+p, f];  kq[p, j, g, f] = kern[j*P+p, f, g]
    x16 = x.astype(np.float16).transpose(1, 0, 2).reshape(jn, P, B, F)
    xq = np.ascontiguousarray(x16.transpose(1, 0, 2, 3)).reshape(P, jn * B * F)
    k16 = kern.astype(np.float16).transpose(0, 2, 1).reshape(jn, P, G, F)
    kq = np.ascontiguousarray(k16.transpose(1, 0, 2, 3)).reshape(P, jn * G * F)

    A16 = A.astype(np.float16)
    in_maps = []
    for r in range(ncores):
        sl = slice(r * nl, (r + 1) * nl)
        # at[p, j, :] = A^T[j*P + p, r-shard] = A[shard, j*P+p].T
        at = np.ascontiguousarray(
            A16[sl, :].T.reshape(jn, P, nl).transpose(1, 0, 2)
        )
        # bsT[(b g), nl] = bias[shard][nl, g] for both b
        bT = np.ascontiguousarray(bias[sl].T)  # [G, nl]
        bsT = np.ascontiguousarray(np.tile(bT, (B, 1))).astype(np.float16)  # [C, nl]
        in_maps.append({"at": at, "xq": xq, "kq": kq, "bsT": bsT})
    return in_maps


def assemble_out(results, n=N, ncores=NCORES):
    nl = n // ncores
    parts = []
    for r in range(ncores):
        o = results[r]["outs"].reshape(B, G, nl)
        parts.append(o.transpose(0, 2, 1))  # [B, nl, G]
    return np.ascontiguousarray(np.concatenate(parts, axis=1))


def run(inputs, n=N, ncores=NCORES, trace=False, **spmd_kwargs):
    from concourse.bass_utils import run_bass_kernel_spmd

    x = np.asarray(inputs["x"], dtype=np.float32)
    A = np.asarray(inputs["A"], dtype=np.float32)
    kern = np.asarray(inputs["kernel"], dtype=np.float32)
    bias = np.asarray(inputs["bias"], dtype=np.float32)
    nc = _get_nc(n, ncores)
    in_maps = make_in_maps(x, A, kern, bias, n, ncores)
    res = run_bass_kernel_spmd(
        nc, in_maps, list(range(ncores)), trace=trace, **spmd_kwargs
    )
    out = assemble_out(res.results, n, ncores)
    return out, res


def kernel(**inputs) -> np.ndarray:
    out, _ = run(inputs)
    return out
